# revision 2
# baseline (speedup 1.0000x reference)
"""Fallback GAT kernel — phase 2 via per-column [128,1] indirect DMA gathers
(the XLA-validated qPoolDynamic shape), cce-add fusing el+er.

Slower (descriptor/instruction bound) but uses only validated primitives.
"""
import numpy as np

from concourse import bass, mybir
import concourse.bacc as bacc
import concourse.tile as tile
import concourse.bass_utils as bass_utils

N = 100000
E = 3200000
K = 8
KD = 512
NCORES = 8
NS = N // NCORES
EC = E // NCORES
P = 128

CTILE = 128                      # idx columns per tile (128 edges each)
ECHUNK = P * CTILE               # 16384 edges per tile
NFULL = EC // ECHUNK             # 24
REM = EC - NFULL * ECHUNK        # 6784 = 128 * 53
REMCOLS = REM // P

f32 = mybir.dt.float32
i32 = mybir.dt.int32


def _make_nc():
    return bacc.Bacc(
        "TRN2",
        target_bir_lowering=False,
        debug=False,
        enable_asserts=False,
        num_devices=NCORES,
    )


def _build_phase1():
    nc = _make_nc()
    feat_src = nc.dram_tensor("feat_src", [NS, KD], f32, kind="ExternalInput").ap()
    feat_dst = nc.dram_tensor("feat_dst", [NS, KD], f32, kind="ExternalInput").ap()
    attn_l = nc.dram_tensor("attn_l", [1, KD], f32, kind="ExternalInput").ap()
    attn_r = nc.dram_tensor("attn_r", [1, KD], f32, kind="ExternalInput").ap()
    el = nc.dram_tensor("el", [NS, K], f32, kind="ExternalOutput").ap()
    er = nc.dram_tensor("er", [NS, K], f32, kind="ExternalOutput").ap()
    with tile.TileContext(nc) as tc:
        with tc.tile_pool(name="sbuf", bufs=4) as pool:
            al = pool.tile([P, KD], f32, tag="attn_l")
            ar = pool.tile([P, KD], f32, tag="attn_r")
            nc.sync.dma_start(out=al[:], in_=attn_l[0:1, :].to_broadcast([P, KD]))
            nc.sync.dma_start(out=ar[:], in_=attn_r[0:1, :].to_broadcast([P, KD]))
            for ti, s in enumerate(range(0, NS, P)):
                p = min(P, NS - s)
                for feat, attn_t, out_d, tag in (
                    (feat_src, al, el, "s"),
                    (feat_dst, ar, er, "d"),
                ):
                    f = pool.tile([P, KD], f32, tag=f"feat{tag}")
                    nc.sync.dma_start(out=f[:p], in_=feat[s : s + p, :])
                    prod = pool.tile([P, KD], f32, tag=f"prod{tag}")
                    eng = nc.gpsimd if (ti % 2 == 0) else nc.vector
                    eng.tensor_tensor(
                        out=prod[:p], in0=f[:p], in1=attn_t[:p],
                        op=mybir.AluOpType.mult,
                    )
                    ot = pool.tile([P, K], f32, tag=f"o{tag}")
                    nc.vector.tensor_reduce(
                        out=ot[:p],
                        in_=prod[:p].rearrange("p (k d) -> p k d", k=K),
                        axis=mybir.AxisListType.X,
                        op=mybir.AluOpType.add,
                    )
                    nc.sync.dma_start(out=out_d[s : s + p, :], in_=ot[:p])
    nc.compile()
    return nc


def _build_phase2():
    nc = _make_nc()
    el = nc.dram_tensor("el", [N, K], f32, kind="ExternalInput").ap()
    er = nc.dram_tensor("er", [N, K], f32, kind="ExternalInput").ap()
    sidx = nc.dram_tensor("sidx", [EC], i32, kind="ExternalInput").ap()
    didx = nc.dram_tensor("didx", [EC], i32, kind="ExternalInput").ap()
    out = nc.dram_tensor("out", [EC, K], f32, kind="ExternalOutput").ap()

    chunks = [(t * ECHUNK, CTILE) for t in range(NFULL)]
    if REM:
        chunks.append((NFULL * ECHUNK, REMCOLS))

    with tile.TileContext(nc) as tc:
        with tc.tile_pool(name="sbuf", bufs=3) as pool:
            for base, cols in chunks:
                st = pool.tile([P, cols], i32, tag="sidx")
                dt_ = pool.tile([P, cols], i32, tag="didx")
                nc.sync.dma_start(
                    out=st[:],
                    in_=sidx[base : base + P * cols].rearrange("(p c) -> p c", p=P),
                )
                nc.scalar.dma_start(
                    out=dt_[:],
                    in_=didx[base : base + P * cols].rearrange("(p c) -> p c", p=P),
                )
                ot = pool.tile([P, cols * K], f32, tag="out")
                for c in range(cols):
                    nc.gpsimd.indirect_dma_start(
                        out=ot[:, c * K : (c + 1) * K],
                        out_offset=None,
                        in_=el[:],
                        in_offset=bass.IndirectOffsetOnAxis(
                            ap=st[:, c : c + 1], axis=0
                        ),
                    )
                    nc.gpsimd.indirect_dma_start(
                        out=ot[:, c * K : (c + 1) * K],
                        out_offset=None,
                        in_=er[:],
                        in_offset=bass.IndirectOffsetOnAxis(
                            ap=dt_[:, c : c + 1], axis=0
                        ),
                        compute_op=mybir.AluOpType.add,
                    )
                nc.sync.dma_start(
                    out=out[base : base + P * cols, :].rearrange(
                        "(p c) k -> p (c k)", p=P
                    ),
                    in_=ot[:],
                )
    nc.compile()
    return nc


_CACHE = {}


def _get_programs():
    if "p1" not in _CACHE:
        _CACHE["p1"] = _build_phase1()
        _CACHE["p2"] = _build_phase2()
    return _CACHE["p1"], _CACHE["p2"]


def kernel(feat_src, feat_dst, attn_l, attn_r, src_idx, dst_idx):
    import time

    feat_src = np.ascontiguousarray(np.asarray(feat_src)).reshape(N, KD)
    feat_dst = np.ascontiguousarray(np.asarray(feat_dst)).reshape(N, KD)
    attn_l = np.ascontiguousarray(np.asarray(attn_l)).reshape(1, KD)
    attn_r = np.ascontiguousarray(np.asarray(attn_r)).reshape(1, KD)
    src_idx = np.ascontiguousarray(np.asarray(src_idx))
    dst_idx = np.ascontiguousarray(np.asarray(dst_idx))

    p1, p2 = _get_programs()
    walls = []

    in_maps1 = [
        {
            "feat_src": feat_src[c * NS : (c + 1) * NS],
            "feat_dst": feat_dst[c * NS : (c + 1) * NS],
            "attn_l": attn_l,
            "attn_r": attn_r,
        }
        for c in range(NCORES)
    ]
    t0 = time.perf_counter()
    r1 = bass_utils.run_bass_kernel_spmd(p1, in_maps1, core_ids=list(range(NCORES)))
    walls.append(time.perf_counter() - t0)
    el = np.concatenate([r1.results[c]["el"] for c in range(NCORES)], axis=0)
    er = np.concatenate([r1.results[c]["er"] for c in range(NCORES)], axis=0)

    in_maps2 = [
        {
            "el": el,
            "er": er,
            "sidx": src_idx[c * EC : (c + 1) * EC],
            "didx": dst_idx[c * EC : (c + 1) * EC],
        }
        for c in range(NCORES)
    ]
    t0 = time.perf_counter()
    r2 = bass_utils.run_bass_kernel_spmd(p2, in_maps2, core_ids=list(range(NCORES)))
    walls.append(time.perf_counter() - t0)
    out = np.concatenate([r2.results[c]["out"] for c in range(NCORES)], axis=0)
    kernel._last_results = (r1, r2)
    kernel._last_phase_walls = walls
    return out.reshape(E, K, 1)
